# revision 1
# baseline (speedup 1.0000x reference)
"""Trainium2 Bass kernel for the per-feature MLP ensemble (dense_mlp).

Reference computation (per feature f of F=128 independent tiny MLPs):
    h1 = elu(X @ W1[f] + b1[f])        X:[N,160]  W1[f]:[160,32]
    h2 = elu(h1 @ W2[f] + b2[f])       W2[f]:[32,32]
    out[:, f] = h2 @ W3[f] + b3[f]     W3[f]:[32]

Strategy:
  - Data-parallel: shard N=32768 rows across 8 cores (4096 each),
    replicate the (tiny) weights.
  - Transposed layout on chip: channels (f,h) on SBUF partitions, n on
    the free dimension.  The F networks are processed in 32 groups of 4
    features = 128 channels, so layer 2 is a single 128x128 block-diagonal
    matmul per group and layer 3 a 128x4 matmul per group.
  - ELU via the exact identity   elu(y) + 1 = max(y + 1, min(e^y, 1))
    (valid because e^y >= 1 + y).  With psum holding y+1 (bias row folded
    into the layer-1 matmul), each ELU costs one ScalarE Exp and one
    VectorE fused scalar_tensor_tensor op:
        u = (e  min  1.0)  max  psum        # u = elu(y)+1
    The +1 offsets are linear, so they fold into the next layer's bias.
"""

import numpy as np

import concourse.bass as bass
import concourse.bacc as bacc
import concourse.mybir as mybir
import concourse.tile as tile
from concourse.bass_utils import run_bass_kernel_spmd

N, D, F, H = 32768, 160, 128, 32
NCORES = 8
NS = N // NCORES          # rows per core
CH = F * H                # 4096 channels after layer 1
GROUPS = F // 4           # 32 groups of 4 features (=128 channels)
CHUNK = 512               # free-dim (n) tile size
NCHUNKS = NS // CHUNK

FP16 = mybir.dt.float16
F32 = mybir.dt.float32
AF = mybir.ActivationFunctionType
ALU = mybir.AluOpType


def _build_bass():
    nc = bacc.Bacc("TRN2", target_bir_lowering=False, debug=False,
                   num_devices=NCORES)

    def inp(name, shape, dt):
        return nc.dram_tensor(name, shape, dt, kind="ExternalInput").ap()

    xt_a = inp("xt_a", [128, NS], FP16)        # X.T rows 0..127 (shard)
    xt_b = inp("xt_b", [33, NS], FP16)         # X.T rows 128..159 + ones row
    w1a = inp("w1a", [128, CH], FP16)          # W1' rows 0..127
    w1b = inp("w1b", [33, CH], FP16)           # W1' rows 128..159 + (b1+1) row
    w2b = inp("w2b", [128, GROUPS * 128], FP16)  # blockdiag(W2) per group
    w3b = inp("w3b", [128, GROUPS * 128], FP16)  # W3 cols placed at out partition
    c2 = inp("c2", [128, GROUPS], F32)         # b2 - colsum(W2), per channel
    c2p1 = inp("c2p1", [128, GROUPS], F32)     # c2 + 1
    b3pp = inp("b3pp", [128, 1], F32)          # b3 + W3 . c2
    neg1 = inp("neg1", [128, 1], F32)          # constant -1 bias column
    outT = nc.dram_tensor("outT", [128, NS], F32, kind="ExternalOutput").ap()

    from contextlib import ExitStack
    with tile.TileContext(nc) as tc, ExitStack() as ctx:
        wp = ctx.enter_context(tc.tile_pool(name="w", bufs=1))

        def load(ap_dram, shape, dt, tag):
            t = wp.tile(list(shape), dt, tag=tag)
            nc.sync.dma_start(t[:], ap_dram)
            return t

        xt_a_sb = load(xt_a, [128, NS], FP16, "xt_a")
        xt_b_sb = load(xt_b, [33, NS], FP16, "xt_b")
        w1a_sb = load(w1a, [128, CH], FP16, "w1a")
        w1b_sb = load(w1b, [33, CH], FP16, "w1b")
        w2b_sb = load(w2b, [128, GROUPS * 128], FP16, "w2b")
        w3b_sb = load(w3b, [128, GROUPS * 128], FP16, "w3b")
        c2_sb = load(c2, [128, GROUPS], F32, "c2")
        c2p1_sb = load(c2p1, [128, GROUPS], F32, "c2p1")
        b3_sb = load(b3pp, [128, 1], F32, "b3pp")
        neg1_sb = load(neg1, [128, 1], F32, "neg1")

        # Warm the ACT Exp table on a tiny tile so the table-load pseudo-op
        # walrus inserts before the first Exp lands on a low-dependency
        # instruction instead of the first real activation.
        warm = wp.tile([128, 1], FP16, tag="warm")
        nc.scalar.activation(warm[:], neg1_sb[:], AF.Exp,
                             bias=neg1_sb[:, 0:1])

        ip = ctx.enter_context(tc.tile_pool(name="interm", bufs=3))
        p1 = ctx.enter_context(tc.tile_pool(name="p1", bufs=2, space="PSUM"))
        p2 = ctx.enter_context(tc.tile_pool(name="p2", bufs=2, space="PSUM"))
        po = ctx.enter_context(tc.tile_pool(name="po", bufs=2, space="PSUM"))
        op = ctx.enter_context(tc.tile_pool(name="osb", bufs=2))

        for ci in range(NCHUNKS):
            cs = slice(ci * CHUNK, (ci + 1) * CHUNK)
            pout = po.tile([128, CHUNK], F32)
            for g in range(GROUPS):
                g128 = slice(128 * g, 128 * (g + 1))
                # ---- layer 1: psum1 = z1 + 1 (bias+1 baked into ones row)
                ps1 = p1.tile([128, CHUNK], F32)
                nc.tensor.matmul(ps1[:], w1a_sb[:, g128], xt_a_sb[:, cs],
                                 start=True, stop=False)
                nc.tensor.matmul(ps1[:], w1b_sb[:, g128], xt_b_sb[:, cs],
                                 start=False, stop=True)
                e1 = ip.tile([128, CHUNK], FP16, tag="e1")
                nc.scalar.activation(e1[:], ps1[:], AF.Exp,
                                     bias=neg1_sb[:, 0:1])
                u1 = ip.tile([128, CHUNK], FP16, tag="u1")
                nc.vector.scalar_tensor_tensor(
                    u1[:], e1[:], 1.0, ps1[:], ALU.min, ALU.max)
                # ---- layer 2: psum2 = z2 - c2
                ps2 = p2.tile([128, CHUNK], F32)
                nc.tensor.matmul(ps2[:], w2b_sb[:, g128], u1[:],
                                 start=True, stop=True)
                e2 = ip.tile([128, CHUNK], FP16, tag="e2")
                nc.scalar.activation(e2[:], ps2[:], AF.Exp,
                                     bias=c2_sb[:, g:g + 1])
                v2 = ip.tile([128, CHUNK], FP16, tag="v2")
                nc.gpsimd.tensor_scalar(
                    v2[:], e2[:], 1.0, c2p1_sb[:, g:g + 1],
                    ALU.min, ALU.subtract)
                u2 = ip.tile([128, CHUNK], FP16, tag="u2")
                nc.vector.scalar_tensor_tensor(
                    u2[:], v2[:], 0.0, ps2[:], ALU.add, ALU.max)
                # ---- layer 3: accumulate all groups into one [128,CHUNK]
                # psum tile; group g's lhsT has its 4 features' W3 in
                # columns 4g..4g+3, zeros elsewhere.
                nc.tensor.matmul(pout[:], w3b_sb[:, g128], u2[:],
                                 start=(g == 0), stop=(g == GROUPS - 1))
            osb = op.tile([128, CHUNK], F32)
            nc.vector.tensor_scalar(osb[:], pout[:], b3_sb[:, 0:1], None,
                                    ALU.add)
            nc.sync.dma_start(outT[:, cs], osb[:])
    nc.compile()
    return nc


def _prep_inputs(X, W1, b1, W2, b2, W3, b3):
    X = np.asarray(X, np.float32)
    W1 = np.asarray(W1, np.float32)
    b1 = np.asarray(b1, np.float32)
    W2 = np.asarray(W2, np.float32)
    b2 = np.asarray(b2, np.float32)
    W3 = np.asarray(W3, np.float32)
    b3 = np.asarray(b3, np.float32)

    W1p = W1.transpose(1, 0, 2).reshape(D, CH)
    b1p = b1.reshape(CH)
    w1a = np.ascontiguousarray(W1p[0:128]).astype(np.float16)
    w1b = np.concatenate([W1p[128:160], (b1p + 1.0)[None, :]], 0).astype(np.float16)

    XT = X.T
    xt_a_full = np.ascontiguousarray(XT[0:128]).astype(np.float16)
    xt_b_full = np.concatenate(
        [XT[128:160], np.ones((1, N), np.float32)], 0).astype(np.float16)

    w2blk = np.zeros((128, GROUPS * 128), np.float32)
    for g in range(GROUPS):
        for j in range(4):
            f = 4 * g + j
            w2blk[32 * j:32 * (j + 1),
                  128 * g + 32 * j:128 * g + 32 * (j + 1)] = W2[f]
    w2blk = w2blk.astype(np.float16)

    colsum2 = W2.sum(axis=1)                       # [F, H]
    c2_ch = (b2 - colsum2).reshape(CH)
    c2 = np.ascontiguousarray(c2_ch.reshape(GROUPS, 128).T).astype(np.float32)
    c2p1 = (c2 + 1.0).astype(np.float32)

    w3blk = np.zeros((128, GROUPS * 128), np.float32)
    for g in range(GROUPS):
        for j in range(4):
            f = 4 * g + j
            w3blk[32 * j:32 * (j + 1), 128 * g + f] = W3[f]
    w3blk = w3blk.astype(np.float16)

    b3pp = (b3 + (W3 * c2_ch.reshape(F, H)).sum(1)).astype(np.float32)
    b3pp = b3pp.reshape(128, 1)
    neg1 = np.full((128, 1), -1.0, np.float32)

    shared = dict(w1a=w1a, w1b=w1b, w2b=w2blk, w3b=w3blk,
                  c2=c2, c2p1=c2p1, b3pp=b3pp, neg1=neg1)
    in_maps = []
    for c in range(NCORES):
        sl = slice(c * NS, (c + 1) * NS)
        m = dict(shared)
        m["xt_a"] = np.ascontiguousarray(xt_a_full[:, sl])
        m["xt_b"] = np.ascontiguousarray(xt_b_full[:, sl])
        in_maps.append(m)
    return in_maps


_NC_CACHE = {}


def _get_nc():
    if "nc" not in _NC_CACHE:
        _NC_CACHE["nc"] = _build_bass()
    return _NC_CACHE["nc"]


def kernel(X, W1, b1, W2, b2, W3, b3, trace=False, trace_kwargs=None):
    nc = _get_nc()
    in_maps = _prep_inputs(X, W1, b1, W2, b2, W3, b3)
    res = run_bass_kernel_spmd(nc, in_maps, list(range(NCORES)),
                               trace=trace, **(trace_kwargs or {}))
    outs = res.results
    outT = np.concatenate([outs[c]["outT"] for c in range(NCORES)], axis=1)
    out = np.ascontiguousarray(outT.T).astype(np.float32)
    if trace:
        kernel.last_results = res
    return out



# revision 8
# speedup vs baseline: 2.3652x; 2.3652x over previous
"""Trainium2 Bass kernel for the per-feature MLP ensemble (dense_mlp).

Reference computation (per feature f of F=128 independent tiny MLPs):
    h1 = elu(X @ W1[f] + b1[f])        X:[N,160]  W1[f]:[160,32]
    h2 = elu(h1 @ W2[f] + b2[f])       W2[f]:[32,32]
    out[:, f] = h2 @ W3[f] + b3[f]     W3[f]:[32]

Strategy:
  - Data-parallel: shard N=32768 rows across 8 cores (4096 each),
    replicate the (tiny) weights.
  - Transposed layout on chip: channels (f,h) on SBUF partitions, n on
    the free dimension.  The F networks are processed in 32 groups of 4
    features = 128 channels, so layer 2 is a single 128x128 block-diagonal
    matmul per group and layer 3 a 128x4 matmul per group.
  - ELU via the exact identity   elu(y) + 1 = max(y + 1, min(e^y, 1))
    (valid because e^y >= 1 + y).  Both layers keep psum = y + 1 (the
    bias+1 row is folded into the matmul: layer 1 via the ones row of
    xt_b, layer 2 via an extra K=1 accumulating matmul whose lhsT is the
    per-channel constant c2+1 and whose rhs is the same ones row).  Each
    ELU then costs exactly one ScalarE Exp (constant bias -1) and one
    VectorE fused scalar_tensor_tensor op:
        u = (e  min  1.0)  max  psum        # u = elu(y)+1
    The +1 offsets are linear, so they fold into the next layer's bias.
  - Software pipelining: the (chunk, group) loop is flattened and split
    into stages L1 / L2 / L3 with offsets 0 / -2 / -4 so the PE queue
    always has independent matmuls ahead of the ones that wait on
    ACT/DVE results (the engine queues are strict FIFO).
"""

import numpy as np

import concourse.bass as bass
import concourse.bacc as bacc
import concourse.mybir as mybir
import concourse.tile as tile
from concourse.bass_utils import run_bass_kernel_spmd

N, D, F, H = 32768, 160, 128, 32
NCORES = 8
NS = N // NCORES          # rows per core
CH = F * H                # 4096 channels after layer 1
GROUPS = F // 4           # 32 groups of 4 features (=128 channels)
CHUNK = 512               # free-dim (n) tile size
NCHUNKS = NS // CHUNK
T = NCHUNKS * GROUPS      # flattened (chunk, group) iteration count

FP16 = mybir.dt.float16
F32 = mybir.dt.float32
AF = mybir.ActivationFunctionType
ALU = mybir.AluOpType


def _build_bass():
    nc = bacc.Bacc("TRN2", target_bir_lowering=False, debug=False,
                   num_devices=NCORES)

    def inp(name, shape, dt):
        return nc.dram_tensor(name, shape, dt, kind="ExternalInput").ap()

    xt_a = inp("xt_a", [128, NS], FP16)        # X.T rows 0..127 (shard)
    xt_b = inp("xt_b", [33, NS], FP16)         # X.T rows 128..159 + ones row
    w1a = inp("w1a", [128, CH], FP16)          # W1' rows 0..127
    w1b = inp("w1b", [33, CH], FP16)           # W1' rows 128..159 + (b1+1) row
    w2b = inp("w2b", [128, GROUPS * 128], FP16)  # blockdiag(W2) per group
    c2p1r = inp("c2p1r", [1, GROUPS * 128], FP16)  # b2 - colsum(W2) + 1
    w3b = inp("w3b", [128, GROUPS * 128], FP16)  # W3 cols placed at out partition
    b3pp = inp("b3pp", [128, 1], F32)          # b3 - rowsum(W3)
    neg1 = inp("neg1", [128, 1], F32)          # constant -1 bias column
    ones = inp("ones", [1, CHUNK], FP16)       # ones rhs for the K=1 bias mm
    outT = nc.dram_tensor("outT", [128, NS], F32, kind="ExternalOutput").ap()

    from contextlib import ExitStack
    with tile.TileContext(nc) as tc, ExitStack() as ctx:
        wp = ctx.enter_context(tc.tile_pool(name="w", bufs=1))

        def load(ap_dram, shape, dt, tag):
            t = wp.tile(list(shape), dt, tag=tag)
            nc.sync.dma_start(t[:], ap_dram)
            return t

        xt_a_sb = load(xt_a, [128, NS], FP16, "xt_a")
        xt_b_sb = load(xt_b, [33, NS], FP16, "xt_b")
        w1a_sb = load(w1a, [128, CH], FP16, "w1a")
        w1b_sb = load(w1b, [33, CH], FP16, "w1b")
        w2b_sb = load(w2b, [128, GROUPS * 128], FP16, "w2b")
        c2_sb = load(c2p1r, [1, GROUPS * 128], FP16, "c2p1r")
        w3b_sb = load(w3b, [128, GROUPS * 128], FP16, "w3b")
        b3_sb = load(b3pp, [128, 1], F32, "b3pp")
        neg1_sb = load(neg1, [128, 1], F32, "neg1")
        ones_sb = load(ones, [1, CHUNK], FP16, "ones")

        # Warm the ACT Exp table on a tiny tile so the table-load pseudo-op
        # lands on a low-dependency instruction instead of the first real
        # activation.
        warm = wp.tile([128, 1], FP16, tag="warm")
        nc.scalar.activation(warm[:], neg1_sb[:], AF.Exp,
                             bias=neg1_sb[:, 0:1])

        ip = ctx.enter_context(tc.tile_pool(name="interm", bufs=4))
        p1 = ctx.enter_context(tc.tile_pool(name="p1", bufs=2, space="PSUM"))
        p2 = ctx.enter_context(tc.tile_pool(name="p2", bufs=2, space="PSUM"))
        po = ctx.enter_context(tc.tile_pool(name="po", bufs=2, space="PSUM"))
        op = ctx.enter_context(tc.tile_pool(name="osb", bufs=2))

        # Pipelined stages over the flat (chunk, group) index t:
        #   L1 at t, L2 at t-2, L3 at t-4.
        u1_tiles = {}   # t -> u1 tile (consumed by L2 at t+2)
        u2_tiles = {}   # t -> u2 tile (consumed by L3 at t+2 of L2 = t+4)
        pout_tiles = {}  # chunk -> accumulating PSUM tile

        def cs_of(t):
            ci = t // GROUPS
            return slice(ci * CHUNK, (ci + 1) * CHUNK)

        def g128_of(t):
            g = t % GROUPS
            return slice(128 * g, 128 * (g + 1))

        for t in range(T + 4):
            # ---------------- L1 stage (group t) ----------------
            if t < T:
                cs, g128 = cs_of(t), g128_of(t)
                ps1 = p1.tile([128, CHUNK], F32)
                nc.tensor.matmul(ps1[:], w1a_sb[:, g128], xt_a_sb[:, cs],
                                 start=True, stop=False)
                nc.tensor.matmul(ps1[:], w1b_sb[:, g128], xt_b_sb[:, cs],
                                 start=False, stop=True)
                e1 = ip.tile([128, CHUNK], FP16, tag="e1")
                nc.scalar.activation(e1[:], ps1[:], AF.Exp,
                                     bias=neg1_sb[:, 0:1])
                u1 = ip.tile([128, CHUNK], FP16, tag="u1")
                nc.vector.scalar_tensor_tensor(
                    u1[:], e1[:], 1.0, ps1[:], ALU.min, ALU.max)
                u1_tiles[t] = u1
            # ---------------- L2 stage (group t-2) ----------------
            s = t - 2
            if 0 <= s < T:
                cs, g128 = cs_of(s), g128_of(s)
                u1 = u1_tiles.pop(s)
                ps2 = p2.tile([128, CHUNK], F32)
                nc.tensor.matmul(ps2[:], w2b_sb[:, g128], u1[:],
                                 start=True, stop=False)
                nc.tensor.matmul(ps2[:], c2_sb[0:1, g128], ones_sb[:],
                                 start=False, stop=True)
                e2 = ip.tile([128, CHUNK], FP16, tag="e2")
                nc.scalar.activation(e2[:], ps2[:], AF.Exp,
                                     bias=neg1_sb[:, 0:1])
                u2 = ip.tile([128, CHUNK], FP16, tag="u2")
                nc.vector.scalar_tensor_tensor(
                    u2[:], e2[:], 1.0, ps2[:], ALU.min, ALU.max)
                u2_tiles[s] = u2
            # ---------------- L3 stage (group t-4) ----------------
            s = t - 4
            if 0 <= s < T:
                ci, g = s // GROUPS, s % GROUPS
                g128 = g128_of(s)
                u2 = u2_tiles.pop(s)
                if g == 0:
                    pout_tiles[ci] = po.tile([128, CHUNK], F32,
                                             name="pout", tag="pout")
                pout = pout_tiles[ci]
                nc.tensor.matmul(pout[:], w3b_sb[:, g128], u2[:],
                                 start=(g == 0), stop=(g == GROUPS - 1))
                if g == GROUPS - 1:
                    cs = cs_of(s)
                    osb = op.tile([128, CHUNK], F32)
                    nc.vector.tensor_scalar(osb[:], pout[:],
                                            b3_sb[:, 0:1], None, ALU.add)
                    nc.sync.dma_start(outT[:, cs], osb[:])
                    del pout_tiles[ci]
    nc.compile()
    return nc


def _prep_inputs(X, W1, b1, W2, b2, W3, b3):
    X = np.asarray(X, np.float32)
    W1 = np.asarray(W1, np.float32)
    b1 = np.asarray(b1, np.float32)
    W2 = np.asarray(W2, np.float32)
    b2 = np.asarray(b2, np.float32)
    W3 = np.asarray(W3, np.float32)
    b3 = np.asarray(b3, np.float32)

    W1p = W1.transpose(1, 0, 2).reshape(D, CH)
    b1p = b1.reshape(CH)
    w1a = np.ascontiguousarray(W1p[0:128]).astype(np.float16)
    w1b = np.concatenate([W1p[128:160], (b1p + 1.0)[None, :]], 0).astype(np.float16)

    XT = X.T
    xt_a_full = np.ascontiguousarray(XT[0:128]).astype(np.float16)
    xt_b_full = np.concatenate(
        [XT[128:160], np.ones((1, N), np.float32)], 0).astype(np.float16)

    w2blk = np.zeros((128, GROUPS * 128), np.float32)
    for g in range(GROUPS):
        for j in range(4):
            f = 4 * g + j
            w2blk[32 * j:32 * (j + 1),
                  128 * g + 32 * j:128 * g + 32 * (j + 1)] = W2[f]
    w2blk = w2blk.astype(np.float16)

    colsum2 = W2.sum(axis=1)                       # [F, H]
    c2_ch = (b2 - colsum2).reshape(CH)             # per-channel c2
    # c2+1 laid out as one row matching the per-group psum partitions
    c2p1r = (c2_ch + 1.0).reshape(1, GROUPS * 128).astype(np.float16)

    w3blk = np.zeros((128, GROUPS * 128), np.float32)
    for g in range(GROUPS):
        for j in range(4):
            f = 4 * g + j
            w3blk[32 * j:32 * (j + 1), 128 * g + f] = W3[f]
    w3blk = w3blk.astype(np.float16)

    # layer-3 rhs is u2 = h2 + 1, so fold -rowsum(W3) into b3
    b3pp = (b3 - W3.sum(axis=1)).astype(np.float32).reshape(128, 1)
    neg1 = np.full((128, 1), -1.0, np.float32)
    ones = np.ones((1, CHUNK), np.float16)

    shared = dict(w1a=w1a, w1b=w1b, w2b=w2blk, c2p1r=c2p1r, w3b=w3blk,
                  b3pp=b3pp, neg1=neg1, ones=ones)
    in_maps = []
    for c in range(NCORES):
        sl = slice(c * NS, (c + 1) * NS)
        m = dict(shared)
        m["xt_a"] = np.ascontiguousarray(xt_a_full[:, sl])
        m["xt_b"] = np.ascontiguousarray(xt_b_full[:, sl])
        in_maps.append(m)
    return in_maps


_NC_CACHE = {}


def _get_nc():
    if "nc" not in _NC_CACHE:
        _NC_CACHE["nc"] = _build_bass()
    return _NC_CACHE["nc"]


def kernel(X, W1, b1, W2, b2, W3, b3, trace=False, trace_kwargs=None):
    nc = _get_nc()
    in_maps = _prep_inputs(X, W1, b1, W2, b2, W3, b3)
    res = run_bass_kernel_spmd(nc, in_maps, list(range(NCORES)),
                               trace=trace, **(trace_kwargs or {}))
    outs = res.results
    outT = np.concatenate([outs[c]["outT"] for c in range(NCORES)], axis=1)
    out = np.ascontiguousarray(outT.T).astype(np.float32)
    if trace:
        kernel.last_results = res
    return out


# revision 13
# speedup vs baseline: 3.0209x; 1.2772x over previous
"""Trainium2 Bass kernel for the per-feature MLP ensemble (dense_mlp).

Reference computation (per feature f of F=128 independent tiny MLPs):
    h1 = elu(X @ W1[f] + b1[f])        X:[N,160]  W1[f]:[160,32]
    h2 = elu(h1 @ W2[f] + b2[f])       W2[f]:[32,32]
    out[:, f] = h2 @ W3[f] + b3[f]     W3[f]:[32]

Strategy:
  - Data-parallel: shard N=32768 rows across 8 cores (4096 each),
    replicate the (tiny) weights.
  - Transposed layout on chip: channels (f,h) on SBUF partitions, n on
    the free dimension.  The F networks are processed in 32 groups of 4
    features = 128 channels, so layer 2 is a single 128x128 block-diagonal
    matmul per group and layer 3 a 128->4 matmul per group.
  - ELU via the exact identity   elu(y) + 1 = max(y + 1, min(e^y, 1)).
    Both layers keep psum = y + 1 (bias+1 folded into the matmul: layer 1
    via the ones row of xt_b, layer 2 via an extra K=1 accumulating
    matmul of the per-channel constant c2+1 against a ones row), so each
    ELU costs one ScalarE Exp (constant -1 bias) and one VectorE
    scalar_tensor_tensor:   u = (e min 1.0) max psum.
  - Chunk-pair iterations: each iteration processes one group x 1024
    rows (two 512 chunks).  Every weight load feeds two back-to-back
    matmuls (adjacent chunks), psum tiles are [128,1024] spanning two
    banks, and ACT/DVE ops run at FD=1024 to amortize per-op overhead.
  - Layer 3 packs each group's 4 output features into a 32-aligned
    partition strip of one shared psum pair via tile_position col
    addressing; 8 groups accumulate per strip.  Global partition index
    equals the feature index, so no output permutation is needed.
  - Software pipelining: stages L1 / L2 / L3 at offsets 0 / -2 / -4 so
    the PE queue always has independent matmuls ahead of the ones that
    wait on ACT/DVE results.
"""

import numpy as np

import concourse.bass as bass
import concourse.bacc as bacc
import concourse.mybir as mybir
import concourse.tile as tile
from concourse.bass_utils import run_bass_kernel_spmd

N, D, F, H = 32768, 160, 128, 32
NCORES = 8
NS = N // NCORES          # rows per core
CH = F * H                # 4096 channels after layer 1
GROUPS = F // 4           # 32 groups of 4 features (=128 channels)
CHUNK = 512               # free-dim (n) sub-tile size (one psum bank)
PAIR = 2 * CHUNK          # rows per iteration
NPAIRS = NS // PAIR
T = NPAIRS * GROUPS       # flattened (pair, group) iteration count

FP16 = mybir.dt.float16
F32 = mybir.dt.float32
AF = mybir.ActivationFunctionType
ALU = mybir.AluOpType


def _build_bass():
    nc = bacc.Bacc("TRN2", target_bir_lowering=False, debug=False,
                   num_devices=NCORES)

    def inp(name, shape, dt):
        return nc.dram_tensor(name, shape, dt, kind="ExternalInput").ap()

    xt_a = inp("xt_a", [128, NS], FP16)        # X.T rows 0..127 (shard)
    xt_b = inp("xt_b", [33, NS], FP16)         # X.T rows 128..159 + ones row
    w1a = inp("w1a", [128, CH], FP16)          # W1' rows 0..127
    w1b = inp("w1b", [33, CH], FP16)           # W1' rows 128..159 + (b1+1) row
    w2b = inp("w2b", [128, GROUPS * 128], FP16)  # blockdiag(W2) per group
    c2p1r = inp("c2p1r", [1, GROUPS * 128], FP16)  # b2 - colsum(W2) + 1
    w3s = inp("w3s", [128, GROUPS * 128], FP16)  # W3 cols at out partition
    b3pp = inp("b3pp", [128, 1], F32)          # b3 - rowsum(W3)
    neg1 = inp("neg1", [128, 1], F32)          # constant -1 bias column
    ones = inp("ones", [1, CHUNK], FP16)       # ones rhs for the K=1 bias mm
    outT = nc.dram_tensor("outT", [128, NS], F32, kind="ExternalOutput").ap()

    from contextlib import ExitStack
    with tile.TileContext(nc) as tc, ExitStack() as ctx:
        wp = ctx.enter_context(tc.tile_pool(name="w", bufs=1))

        def load(ap_dram, shape, dt, tag):
            t = wp.tile(list(shape), dt, tag=tag)
            nc.sync.dma_start(t[:], ap_dram)
            return t

        xt_a_sb = load(xt_a, [128, NS], FP16, "xt_a")
        xt_b_sb = load(xt_b, [33, NS], FP16, "xt_b")
        w1a_sb = load(w1a, [128, CH], FP16, "w1a")
        w1b_sb = load(w1b, [33, CH], FP16, "w1b")
        w2b_sb = load(w2b, [128, GROUPS * 128], FP16, "w2b")
        c2_sb = load(c2p1r, [1, GROUPS * 128], FP16, "c2p1r")
        w3s_sb = load(w3s, [128, GROUPS * 128], FP16, "w3s")
        b3_sb = load(b3pp, [128, 1], F32, "b3pp")
        neg1_sb = load(neg1, [128, 1], F32, "neg1")
        ones_sb = load(ones, [1, CHUNK], FP16, "ones")

        # Warm the ACT Exp table on a tiny tile so the table-load pseudo-op
        # lands on a low-dependency instruction instead of the first real
        # activation.
        warm = wp.tile([128, 1], FP16, tag="warm")
        nc.scalar.activation(warm[:], neg1_sb[:], AF.Exp,
                             bias=neg1_sb[:, 0:1])

        ip = ctx.enter_context(tc.tile_pool(name="interm", bufs=4))
        p1 = ctx.enter_context(tc.tile_pool(name="p1", bufs=2, space="PSUM"))
        p2 = ctx.enter_context(tc.tile_pool(name="p2", bufs=1, space="PSUM"))
        po = ctx.enter_context(tc.tile_pool(name="po", bufs=1, space="PSUM"))
        op = ctx.enter_context(tc.tile_pool(name="osb", bufs=2))

        u1_tiles = {}
        u2_tiles = {}
        pout_tiles = {}

        def halves(t):
            P = t // GROUPS
            a = slice(PAIR * P, PAIR * P + CHUNK)
            b = slice(PAIR * P + CHUNK, PAIR * (P + 1))
            return a, b

        def g128_of(t):
            g = t % GROUPS
            return slice(128 * g, 128 * (g + 1))

        for t in range(T + 4):
            # ---------------- L1 stage (iter t) ----------------
            if t < T:
                ca, cb = halves(t)
                g128 = g128_of(t)
                ps1 = p1.tile([128, PAIR], F32)
                nc.tensor.matmul(ps1[:, 0:CHUNK], w1a_sb[:, g128],
                                 xt_a_sb[:, ca], start=True, stop=False)
                nc.tensor.matmul(ps1[:, CHUNK:PAIR], w1a_sb[:, g128],
                                 xt_a_sb[:, cb], start=True, stop=False)
                nc.tensor.matmul(ps1[:, 0:CHUNK], w1b_sb[:, g128],
                                 xt_b_sb[:, ca], start=False, stop=True)
                nc.tensor.matmul(ps1[:, CHUNK:PAIR], w1b_sb[:, g128],
                                 xt_b_sb[:, cb], start=False, stop=True)
                e1 = ip.tile([128, PAIR], FP16, tag="e1")
                nc.scalar.activation(e1[:], ps1[:], AF.Exp,
                                     bias=neg1_sb[:, 0:1])
                u1 = ip.tile([128, PAIR], FP16, tag="u1")
                nc.vector.scalar_tensor_tensor(
                    u1[:], e1[:], 1.0, ps1[:], ALU.min, ALU.max)
                u1_tiles[t] = u1
            # ---------------- L2 stage (iter t-2) ----------------
            s = t - 2
            if 0 <= s < T:
                g128 = g128_of(s)
                u1 = u1_tiles.pop(s)
                ps2 = p2.tile([128, PAIR], F32)
                nc.tensor.matmul(ps2[:, 0:CHUNK], w2b_sb[:, g128],
                                 u1[:, 0:CHUNK], start=True, stop=False)
                nc.tensor.matmul(ps2[:, CHUNK:PAIR], w2b_sb[:, g128],
                                 u1[:, CHUNK:PAIR], start=True, stop=False)
                nc.tensor.matmul(ps2[:, 0:CHUNK], c2_sb[0:1, g128],
                                 ones_sb[:], start=False, stop=True)
                nc.tensor.matmul(ps2[:, CHUNK:PAIR], c2_sb[0:1, g128],
                                 ones_sb[:], start=False, stop=True)
                e2 = ip.tile([128, PAIR], FP16, tag="e2")
                nc.scalar.activation(e2[:], ps2[:], AF.Exp,
                                     bias=neg1_sb[:, 0:1])
                u2 = ip.tile([128, PAIR], FP16, tag="u2")
                nc.vector.scalar_tensor_tensor(
                    u2[:], e2[:], 1.0, ps2[:], ALU.min, ALU.max)
                u2_tiles[s] = u2
            # ---------------- L3 stage (iter t-4) ----------------
            s = t - 4
            if 0 <= s < T:
                P, g = s // GROUPS, s % GROUPS
                u2 = u2_tiles.pop(s)
                if g == 0:
                    pout_tiles[P] = po.tile([128, PAIR], F32,
                                            name="pout", tag="pout")
                pout = pout_tiles[P]
                w3g = w3s_sb[:, 128 * g:128 * (g + 1)]
                first = (g == 0)
                last = (g == GROUPS - 1)
                nc.tensor.matmul(pout[:, 0:CHUNK], w3g,
                                 u2[:, 0:CHUNK], start=first, stop=last)
                nc.tensor.matmul(pout[:, CHUNK:PAIR], w3g,
                                 u2[:, CHUNK:PAIR], start=first, stop=last)
                if g == GROUPS - 1:
                    osb = op.tile([128, PAIR], F32, name="osb", tag="osb")
                    nc.vector.tensor_scalar(osb[:], pout[:],
                                            b3_sb[:, 0:1], None, ALU.add)
                    nc.sync.dma_start(outT[:, PAIR * P:PAIR * (P + 1)],
                                      osb[:])
                    del pout_tiles[P]
    nc.compile()
    return nc


def _prep_inputs(X, W1, b1, W2, b2, W3, b3):
    X = np.asarray(X, np.float32)
    W1 = np.asarray(W1, np.float32)
    b1 = np.asarray(b1, np.float32)
    W2 = np.asarray(W2, np.float32)
    b2 = np.asarray(b2, np.float32)
    W3 = np.asarray(W3, np.float32)
    b3 = np.asarray(b3, np.float32)

    W1p = W1.transpose(1, 0, 2).reshape(D, CH)
    b1p = b1.reshape(CH)
    w1a = np.ascontiguousarray(W1p[0:128]).astype(np.float16)
    w1b = np.concatenate([W1p[128:160], (b1p + 1.0)[None, :]], 0).astype(np.float16)

    XT = X.T
    xt_a_full = np.ascontiguousarray(XT[0:128]).astype(np.float16)
    xt_b_full = np.concatenate(
        [XT[128:160], np.ones((1, N), np.float32)], 0).astype(np.float16)

    w2blk = np.zeros((128, GROUPS * 128), np.float32)
    for g in range(GROUPS):
        for j in range(4):
            f = 4 * g + j
            w2blk[32 * j:32 * (j + 1),
                  128 * g + 32 * j:128 * g + 32 * (j + 1)] = W2[f]
    w2blk = w2blk.astype(np.float16)

    colsum2 = W2.sum(axis=1)                       # [F, H]
    c2_ch = (b2 - colsum2).reshape(CH)             # per-channel c2
    c2p1r = (c2_ch + 1.0).reshape(1, GROUPS * 128).astype(np.float16)

    w3s = np.zeros((128, GROUPS * 128), np.float32)
    for g in range(GROUPS):
        for j in range(4):
            f = 4 * g + j
            w3s[32 * j:32 * (j + 1), 128 * g + f] = W3[f]
    w3s = w3s.astype(np.float16)

    # layer-3 rhs is u2 = h2 + 1, so fold -rowsum(W3) into b3
    b3pp = (b3 - W3.sum(axis=1)).astype(np.float32).reshape(128, 1)
    neg1 = np.full((128, 1), -1.0, np.float32)
    ones = np.ones((1, CHUNK), np.float16)

    shared = dict(w1a=w1a, w1b=w1b, w2b=w2blk, c2p1r=c2p1r, w3s=w3s,
                  b3pp=b3pp, neg1=neg1, ones=ones)
    in_maps = []
    for c in range(NCORES):
        sl = slice(c * NS, (c + 1) * NS)
        m = dict(shared)
        m["xt_a"] = np.ascontiguousarray(xt_a_full[:, sl])
        m["xt_b"] = np.ascontiguousarray(xt_b_full[:, sl])
        in_maps.append(m)
    return in_maps


_NC_CACHE = {}


def _get_nc():
    if "nc" not in _NC_CACHE:
        _NC_CACHE["nc"] = _build_bass()
    return _NC_CACHE["nc"]


def kernel(X, W1, b1, W2, b2, W3, b3, trace=False, trace_kwargs=None):
    nc = _get_nc()
    in_maps = _prep_inputs(X, W1, b1, W2, b2, W3, b3)
    res = run_bass_kernel_spmd(nc, in_maps, list(range(NCORES)),
                               trace=trace, **(trace_kwargs or {}))
    outs = res.results
    outT = np.concatenate([outs[c]["outT"] for c in range(NCORES)], axis=1)
    out = np.ascontiguousarray(outT.T).astype(np.float32)
    if trace:
        kernel.last_results = res
    return out


# revision 16
# speedup vs baseline: 3.0234x; 1.0008x over previous
"""Trainium2 Bass kernel for the per-feature MLP ensemble (dense_mlp).

Reference computation (per feature f of F=128 independent tiny MLPs):
    h1 = elu(X @ W1[f] + b1[f])        X:[N,160]  W1[f]:[160,32]
    h2 = elu(h1 @ W2[f] + b2[f])       W2[f]:[32,32]
    out[:, f] = h2 @ W3[f] + b3[f]     W3[f]:[32]

Strategy:
  - Data-parallel: shard N=32768 rows across 8 cores (4096 each),
    replicate the (tiny) weights.
  - Transposed layout on chip: channels (f,h) on SBUF partitions, n on
    the free dimension.  The F networks are processed in 32 groups of 4
    features = 128 channels, so layer 2 is a single 128x128 block-diagonal
    matmul per group and layer 3 a 128->4 matmul per group.
  - ELU via the exact identity   elu(y) + 1 = max(y + 1, min(e^y, 1)).
    Both layers keep psum = y + 1 (bias+1 folded into the matmul: layer 1
    via the ones row of xt_b, layer 2 via an extra K=1 accumulating
    matmul of the per-channel constant c2+1 against a ones row), so each
    ELU costs one ScalarE Exp (constant -1 bias) and one VectorE
    scalar_tensor_tensor:   u = (e min 1.0) max psum.
  - Chunk-pair iterations: each iteration processes one group x 1024
    rows (two 512 chunks).  Every weight load feeds two back-to-back
    matmuls (adjacent chunks), psum tiles are [128,1024] spanning two
    banks, and ACT/DVE ops run at FD=1024 to amortize per-op overhead.
  - Layer 3 packs each group's 4 output features into a 32-aligned
    partition strip of one shared psum pair via tile_position col
    addressing; 8 groups accumulate per strip.  Global partition index
    equals the feature index, so no output permutation is needed.
  - Software pipelining: stages L1 / L2 / L3 at offsets 0 / -2 / -4 so
    the PE queue always has independent matmuls ahead of the ones that
    wait on ACT/DVE results.
"""

import numpy as np

import concourse.bass as bass
import concourse.bacc as bacc
import concourse.mybir as mybir
import concourse.tile as tile
from concourse.bass_utils import run_bass_kernel_spmd

N, D, F, H = 32768, 160, 128, 32
NCORES = 8
NS = N // NCORES          # rows per core
CH = F * H                # 4096 channels after layer 1
GROUPS = F // 4           # 32 groups of 4 features (=128 channels)
CHUNK = 512               # free-dim (n) sub-tile size (one psum bank)
PAIR = 2 * CHUNK          # rows per iteration
NPAIRS = NS // PAIR
T = NPAIRS * GROUPS       # flattened (pair, group) iteration count

FP16 = mybir.dt.float16
F32 = mybir.dt.float32
AF = mybir.ActivationFunctionType
ALU = mybir.AluOpType


def _build_bass():
    nc = bacc.Bacc("TRN2", target_bir_lowering=False, debug=False,
                   num_devices=NCORES)

    def inp(name, shape, dt):
        return nc.dram_tensor(name, shape, dt, kind="ExternalInput").ap()

    xt_a = inp("xt_a", [128, NS], FP16)        # X.T rows 0..127 (shard)
    xt_b = inp("xt_b", [33, NS], FP16)         # X.T rows 128..159 + ones row
    w1a = inp("w1a", [128, CH], FP16)          # W1' rows 0..127
    w1b = inp("w1b", [33, CH], FP16)           # W1' rows 128..159 + (b1+1) row
    w2b = inp("w2b", [128, GROUPS * 128], FP16)  # blockdiag(W2) per group
    c2p1r = inp("c2p1r", [1, GROUPS * 128], FP16)  # b2 - colsum(W2) + 1
    w3s = inp("w3s", [128, GROUPS * 128], FP16)  # W3 cols at out partition
    b3pp = inp("b3pp", [128, 1], F32)          # b3 - rowsum(W3)
    neg1 = inp("neg1", [128, 1], F32)          # constant -1 bias column
    ones = inp("ones", [1, CHUNK], FP16)       # ones rhs for the K=1 bias mm
    outT = nc.dram_tensor("outT", [128, NS], F32, kind="ExternalOutput").ap()

    from contextlib import ExitStack
    with tile.TileContext(nc) as tc, ExitStack() as ctx:
        wp = ctx.enter_context(tc.tile_pool(name="w", bufs=1))

        def load(ap_dram, shape, dt, tag):
            t = wp.tile(list(shape), dt, tag=tag)
            nc.sync.dma_start(t[:], ap_dram)
            return t

        xt_a_sb = load(xt_a, [128, NS], FP16, "xt_a")
        xt_b_sb = load(xt_b, [33, NS], FP16, "xt_b")
        w1a_sb = load(w1a, [128, CH], FP16, "w1a")
        w1b_sb = load(w1b, [33, CH], FP16, "w1b")
        w2b_sb = load(w2b, [128, GROUPS * 128], FP16, "w2b")
        c2_sb = load(c2p1r, [1, GROUPS * 128], FP16, "c2p1r")
        w3s_sb = load(w3s, [128, GROUPS * 128], FP16, "w3s")
        b3_sb = load(b3pp, [128, 1], F32, "b3pp")
        neg1_sb = load(neg1, [128, 1], F32, "neg1")
        ones_sb = load(ones, [1, CHUNK], FP16, "ones")

        # Warm the ACT Exp table on a tiny tile so the table-load pseudo-op
        # lands on a low-dependency instruction instead of the first real
        # activation.
        warm = wp.tile([128, 1], FP16, tag="warm")
        nc.scalar.activation(warm[:], neg1_sb[:], AF.Exp,
                             bias=neg1_sb[:, 0:1])

        ip = ctx.enter_context(tc.tile_pool(name="interm", bufs=4))
        p1 = ctx.enter_context(tc.tile_pool(name="p1", bufs=2, space="PSUM"))
        p2 = ctx.enter_context(tc.tile_pool(name="p2", bufs=1, space="PSUM"))
        po = ctx.enter_context(tc.tile_pool(name="po", bufs=1, space="PSUM"))
        op = ctx.enter_context(tc.tile_pool(name="osb", bufs=2))

        u1_tiles = {}
        u2_tiles = {}
        pout_tiles = {}

        def halves(t):
            P = t // GROUPS
            a = slice(PAIR * P, PAIR * P + CHUNK)
            b = slice(PAIR * P + CHUNK, PAIR * (P + 1))
            return a, b

        def g128_of(t):
            g = t % GROUPS
            return slice(128 * g, 128 * (g + 1))

        for t in range(T + 4):
            # ---------------- L1 stage (iter t) ----------------
            if t < T:
                ca, cb = halves(t)
                g128 = g128_of(t)
                ps1 = p1.tile([128, PAIR], F32)
                nc.tensor.matmul(ps1[:, 0:CHUNK], w1a_sb[:, g128],
                                 xt_a_sb[:, ca], start=True, stop=False)
                mm = nc.tensor.matmul(ps1[:, CHUNK:PAIR], w1a_sb[:, g128],
                                      xt_a_sb[:, cb], start=True, stop=False)
                mm.ins.ldweights = False
                nc.tensor.matmul(ps1[:, 0:CHUNK], w1b_sb[:, g128],
                                 xt_b_sb[:, ca], start=False, stop=True)
                mm = nc.tensor.matmul(ps1[:, CHUNK:PAIR], w1b_sb[:, g128],
                                      xt_b_sb[:, cb], start=False, stop=True)
                mm.ins.ldweights = False
                e1 = ip.tile([128, PAIR], FP16, tag="e1")
                nc.scalar.activation(e1[:], ps1[:], AF.Exp,
                                     bias=neg1_sb[:, 0:1])
                u1 = ip.tile([128, PAIR], FP16, tag="u1")
                nc.vector.scalar_tensor_tensor(
                    u1[:], e1[:], 1.0, ps1[:], ALU.min, ALU.max)
                u1_tiles[t] = u1
            # ---------------- L2 stage (iter t-2) ----------------
            s = t - 2
            if 0 <= s < T:
                g128 = g128_of(s)
                u1 = u1_tiles.pop(s)
                ps2 = p2.tile([128, PAIR], F32)
                nc.tensor.matmul(ps2[:, 0:CHUNK], w2b_sb[:, g128],
                                 u1[:, 0:CHUNK], start=True, stop=False)
                mm = nc.tensor.matmul(ps2[:, CHUNK:PAIR], w2b_sb[:, g128],
                                      u1[:, CHUNK:PAIR], start=True, stop=False)
                mm.ins.ldweights = False
                nc.tensor.matmul(ps2[:, 0:CHUNK], c2_sb[0:1, g128],
                                 ones_sb[:], start=False, stop=True)
                mm = nc.tensor.matmul(ps2[:, CHUNK:PAIR], c2_sb[0:1, g128],
                                      ones_sb[:], start=False, stop=True)
                mm.ins.ldweights = False
                e2 = ip.tile([128, PAIR], FP16, tag="e2")
                nc.scalar.activation(e2[:], ps2[:], AF.Exp,
                                     bias=neg1_sb[:, 0:1])
                u2 = ip.tile([128, PAIR], FP16, tag="u2")
                nc.vector.scalar_tensor_tensor(
                    u2[:], e2[:], 1.0, ps2[:], ALU.min, ALU.max)
                u2_tiles[s] = u2
            # ---------------- L3 stage (iter t-4) ----------------
            s = t - 4
            if 0 <= s < T:
                P, g = s // GROUPS, s % GROUPS
                u2 = u2_tiles.pop(s)
                if g == 0:
                    pout_tiles[P] = po.tile([128, PAIR], F32,
                                            name="pout", tag="pout")
                pout = pout_tiles[P]
                w3g = w3s_sb[:, 128 * g:128 * (g + 1)]
                first = (g == 0)
                last = (g == GROUPS - 1)
                nc.tensor.matmul(pout[:, 0:CHUNK], w3g,
                                 u2[:, 0:CHUNK], start=first, stop=last)
                mm = nc.tensor.matmul(pout[:, CHUNK:PAIR], w3g,
                                      u2[:, CHUNK:PAIR], start=first, stop=last)
                mm.ins.ldweights = False
                if g == GROUPS - 1:
                    osb = op.tile([128, PAIR], F32, name="osb", tag="osb")
                    nc.vector.tensor_scalar(osb[:], pout[:],
                                            b3_sb[:, 0:1], None, ALU.add)
                    nc.sync.dma_start(outT[:, PAIR * P:PAIR * (P + 1)],
                                      osb[:])
                    del pout_tiles[P]
    nc.compile()
    return nc


def _prep_inputs(X, W1, b1, W2, b2, W3, b3):
    X = np.asarray(X, np.float32)
    W1 = np.asarray(W1, np.float32)
    b1 = np.asarray(b1, np.float32)
    W2 = np.asarray(W2, np.float32)
    b2 = np.asarray(b2, np.float32)
    W3 = np.asarray(W3, np.float32)
    b3 = np.asarray(b3, np.float32)

    W1p = W1.transpose(1, 0, 2).reshape(D, CH)
    b1p = b1.reshape(CH)
    w1a = np.ascontiguousarray(W1p[0:128]).astype(np.float16)
    w1b = np.concatenate([W1p[128:160], (b1p + 1.0)[None, :]], 0).astype(np.float16)

    XT = X.T
    xt_a_full = np.ascontiguousarray(XT[0:128]).astype(np.float16)
    xt_b_full = np.concatenate(
        [XT[128:160], np.ones((1, N), np.float32)], 0).astype(np.float16)

    w2blk = np.zeros((128, GROUPS * 128), np.float32)
    for g in range(GROUPS):
        for j in range(4):
            f = 4 * g + j
            w2blk[32 * j:32 * (j + 1),
                  128 * g + 32 * j:128 * g + 32 * (j + 1)] = W2[f]
    w2blk = w2blk.astype(np.float16)

    colsum2 = W2.sum(axis=1)                       # [F, H]
    c2_ch = (b2 - colsum2).reshape(CH)             # per-channel c2
    c2p1r = (c2_ch + 1.0).reshape(1, GROUPS * 128).astype(np.float16)

    w3s = np.zeros((128, GROUPS * 128), np.float32)
    for g in range(GROUPS):
        for j in range(4):
            f = 4 * g + j
            w3s[32 * j:32 * (j + 1), 128 * g + f] = W3[f]
    w3s = w3s.astype(np.float16)

    # layer-3 rhs is u2 = h2 + 1, so fold -rowsum(W3) into b3
    b3pp = (b3 - W3.sum(axis=1)).astype(np.float32).reshape(128, 1)
    neg1 = np.full((128, 1), -1.0, np.float32)
    ones = np.ones((1, CHUNK), np.float16)

    shared = dict(w1a=w1a, w1b=w1b, w2b=w2blk, c2p1r=c2p1r, w3s=w3s,
                  b3pp=b3pp, neg1=neg1, ones=ones)
    in_maps = []
    for c in range(NCORES):
        sl = slice(c * NS, (c + 1) * NS)
        m = dict(shared)
        m["xt_a"] = np.ascontiguousarray(xt_a_full[:, sl])
        m["xt_b"] = np.ascontiguousarray(xt_b_full[:, sl])
        in_maps.append(m)
    return in_maps


_NC_CACHE = {}


def _get_nc():
    if "nc" not in _NC_CACHE:
        _NC_CACHE["nc"] = _build_bass()
    return _NC_CACHE["nc"]


def kernel(X, W1, b1, W2, b2, W3, b3, trace=False, trace_kwargs=None):
    nc = _get_nc()
    in_maps = _prep_inputs(X, W1, b1, W2, b2, W3, b3)
    res = run_bass_kernel_spmd(nc, in_maps, list(range(NCORES)),
                               trace=trace, **(trace_kwargs or {}))
    outs = res.results
    outT = np.concatenate([outs[c]["outT"] for c in range(NCORES)], axis=1)
    out = np.ascontiguousarray(outT.T).astype(np.float32)
    if trace:
        kernel.last_results = res
    return out
